# revision 24
# baseline (speedup 1.0000x reference)
"""DiffusionGraphConv Trainium2 kernel (8-core SPMD, data-parallel over batch).

Math refactoring (halves the big-matmul FLOPs vs the reference order):
  reference: out[b,n,o] = sum_{f,m} mats_m[n,f,b] * W[f*5+m, o]
  with mats = [x0, s0 x0, 2 s0^2 x0 - x0, s1 x0, 2 s1^2 x0 - x0].
  Projection (width F=128 -> O=64) commutes with the node-space diffusion, so:
    u_m = proj(x0, W_m)                       # [N, O, B] each, cheap
    out = (u0 - u2 - u4) + s0 (u1 + 2 s0 u2) + s1 (u3 + 2 s1 u4)
  Device computes: v0 = proj(x0, W0-W2-W4), and with pre-scaled 2*W2 / 2*W4:
    c0 = u1 + s0 @ u2s ; c1 = u3 + s1 @ u4s ; out = v0 + s0 @ c0 + s1 @ c1

Per-core work: 4 matmuls [2048,2048]@[2048,512] (bf16, fp32 PSUM) + projections.
Layouts (host-prepared, all "SBUF images"):
  x0t  [128 f, 8b*2048n]  bf16 : x0t[f, b*N+n] = concat(inputs,state)[b, n, f]
  wcat [128 f, 5*64]      bf16 : [W0-W2-W4 | W1 | 2*W2 | W3 | 2*W4]
  s*t  [16 t, 128 p, 2048] bf16: s*t[t, p, kt*128+j] = s[t*128+j, kt*128+p]
       (strip t = transposed rows of s for output-node tile t, k-major)
  out  [2048 n, 8b*64o]   f32
"""

import sys

if "/opt/trn_rl_repo" not in sys.path:
    sys.path.insert(0, "/opt/trn_rl_repo")

import numpy as np
import ml_dtypes

import concourse.bass as bass
import concourse.mybir as mybir
from concourse.tile import TileContext
from concourse.bass_utils import run_bass_kernel_spmd

BF16 = mybir.dt.bfloat16
F32 = mybir.dt.float32
NPBF16 = ml_dtypes.bfloat16

N = 2048          # graph nodes
F = 128           # input_size (64 input + 64 hidden)
B = 64            # global batch
NCORES = 8
BS = B // NCORES  # 8 batches per core
O = 64            # output features
NT = N // 128     # 16 node tiles
M5 = 5            # diffusion matrices


def _legalize_waits(nc, max_waits=1):
    """Walrus in this env encodes at most one sync-wait per instruction.

    Tile's sem assignment can emit 2-3 waits on one instruction; hoist the
    excess onto standalone EventSemaphore carriers (same engine, inserted
    just before), which the sequencer executes in order — semantics are
    identical, encoding is legal."""
    f = nc.m.functions[0]
    for blk in f.blocks:
        new_insts = []
        changed = False
        for inst in blk.instructions:
            si = inst.sync_info
            waits = list(si.on_wait) if si is not None else []
            if len(waits) > max_waits:
                for i, w in enumerate(waits[:-max_waits]):
                    ev = mybir.InstEventSemaphore(
                        name=f"{inst.name}-wsplit{i}",
                        engine=inst.engine,
                        ins=[],
                        outs=[],
                        sync_info=mybir.SyncInfo(on_wait=[w], on_update=[]),
                    )
                    new_insts.append(ev)
                inst.sync_info = mybir.SyncInfo(
                    on_wait=waits[-max_waits:], on_update=list(si.on_update)
                )
                changed = True
            new_insts.append(inst)
        if changed:
            blk.instructions = new_insts
    return nc


def build_bass(n=N, bs=BS, o=O, legalize=True, n_hops=4, repeat=1):
    """Build the per-core SPMD Bass program."""
    nt = n // 128
    nc = bass.Bass()
    x0t = nc.dram_tensor("x0t", [F, bs * n], BF16, kind="ExternalInput")
    wcat = nc.dram_tensor("wcat", [F, M5 * o], BF16, kind="ExternalInput")
    s0t = nc.dram_tensor("s0t", [nt, 128, n], BF16, kind="ExternalInput")
    s1t = nc.dram_tensor("s1t", [nt, 128, n], BF16, kind="ExternalInput")
    out = nc.dram_tensor("out", [n, bs * o], F32, kind="ExternalOutput")

    obs = bs * o        # 512: width of diffusion operands
    with TileContext(nc) as tc:
        with (
            tc.tile_pool(name="persist", bufs=1) as persist,
            tc.tile_pool(name="stream", bufs=6) as stream,
            tc.tile_pool(name="pproj", bufs=2, space="PSUM") as pproj,
            tc.tile_pool(name="pacc", bufs=4, space="PSUM") as pacc,
        ):
            w_sb = persist.tile([F, M5 * o], BF16, name="w_sb")
            nc.sync.dma_start(out=w_sb[:, :], in_=wcat[:, :])
            # x0t is t-major on host: free index = t*bs*128 + b*128 + j, so
            # each node-tile's stationary slices arrive in one chunk DMA.
            x0_sb = persist.tile([F, bs * n], BF16, name="x0_sb")
            for t in range(nt):
                nc.sync.dma_start(
                    out=x0_sb[:, t * bs * 128:(t + 1) * bs * 128],
                    in_=x0t[:, t * bs * 128:(t + 1) * bs * 128],
                )
            # U[t]: [128, bs*5*o] bf16, b-major: free = b*320 + mi*64 + oo.
            # Slots mi: 0=v0, 1=u1->c0, 2=2*u2, 3=u3->c1, 4=2*u4.
            U = [
                persist.tile([128, bs * M5 * o], BF16, name=f"u{t}", tag=f"u{t}")
                for t in range(nt)
            ]
            # V[t]: [128, obs] f32 accumulator, created in the first V-hop.
            V = [
                persist.tile([128, obs], F32, name=f"v{t}", tag=f"v{t}")
                for t in range(nt)
            ]

            def uslot(t, mi):
                """[128, bs, o] strided moving/elementwise view of slot mi."""
                return U[t].rearrange("p (b mio) -> p b mio", b=bs)[
                    :, :, mi * o:(mi + 1) * o
                ]

            # ---- Phase 1: projections, node-tile outer so U[t] completes
            # early and hop-1 PSUM groups can close while P1 still runs.
            #   psum[:, h*512 : h*512+320] = x0_tile(b).T @ wcat   (n on psum partitions)
            def phase1(t):
                for bp in range(bs // 2):
                    ps = pproj.tile([128, 1024], F32, name="ps_proj", tag="proj")
                    for h in range(2):
                        b = bp * 2 + h
                        nc.tensor.matmul(
                            ps[:, h * 512: h * 512 + M5 * o],
                            lhsT=x0_sb[
                                :, (t * bs + b) * 128:(t * bs + b + 1) * 128
                            ],
                            rhs=w_sb[:, :],
                            start=True,
                            stop=True,
                        )
                    # one combined [128, 2, 320] copy per psum tile (fewer op
                    # overheads + sem round-trips), alternating DVE/ACT so the
                    # copy engines pipeline with PE at ~2x rate
                    b0 = bp * 2
                    dst = U[t][:, b0 * M5 * o:(b0 + 2) * M5 * o].rearrange(
                        "p (hh c) -> p hh c", hh=2
                    )
                    src = ps.rearrange("p (hh c) -> p hh c", hh=2)[:, :, 0:M5 * o]
                    if bp % 2 == 0:
                        nc.vector.tensor_copy(out=dst, in_=src)
                    else:
                        nc.scalar.copy(out=dst, in_=src)

            # ---- Phases 2-5: diffusion hops.
            #   hop(s, src_slot, dst):  for each node-tile t:
            #     acc = sum_kt sT_strip[t,kt].T @ U[kt][src_slot]   (= (s @ u)[t-tile])
            def hop(sdram, src, dst_slot, first_v, final, split_k=1):
                for t in range(nt):
                    strip = stream.tile([128, n], BF16, name="strip", tag="strip")
                    nc.sync.dma_start(out=strip[:, :], in_=sdram[t])
                    # split_k>1: independent psum sub-groups over kt ranges, so
                    # early sub-groups can close while upstream U tiles are
                    # still being produced (fills PE idle at phase boundaries)
                    kchunk = nt // split_k
                    pss = []
                    for g in range(split_k):
                        ps = pacc.tile([128, obs], F32, name="ps_acc", tag="acc")
                        for i, kt in enumerate(range(g * kchunk, (g + 1) * kchunk)):
                            nc.tensor.matmul(
                                ps[:, :],
                                lhsT=strip[:, kt * 128:(kt + 1) * 128],
                                rhs=uslot(kt, src),
                                start=(i == 0),
                                stop=(i == kchunk - 1),
                            )
                        pss.append(ps)
                    if first_v:
                        # V = v0 + s0 @ c0   (V layout: b*o + oo, matches psum)
                        nc.vector.tensor_add(V[t][:, :], pss[0][:, :], uslot(t, 0))
                        for ps in pss[1:]:
                            nc.vector.tensor_add(V[t][:, :], V[t][:, :], ps[:, :])
                    elif final:
                        for ps in pss:
                            nc.vector.tensor_add(V[t][:, :], V[t][:, :], ps[:, :])
                        nc.sync.dma_start(
                            out=out[t * 128:(t + 1) * 128, :], in_=V[t][:, :]
                        )
                    else:
                        d = uslot(t, dst_slot)
                        for ps in pss:
                            nc.vector.tensor_add(d, d, ps[:, :])

            hops = [
                (s0t, 2, 1, False, False, 1),    # c0 = u1 + s0 @ (2 u2)
                (s1t, 4, 3, False, False, 1),    # c1 = u3 + s1 @ (2 u4)
                (s0t, 1, None, True, False, 1),  # V = v0 + s0 @ c0
                (s1t, 3, None, False, True, 1),  # V += s1 @ c1 ; dma out
            ]
            # repeat>1 re-runs the whole idempotent pipeline (each round
            # rebuilds U from x0 and recreates V) — used only to measure
            # per-round device time via wall-clock differencing.
            for _rep in range(repeat):
                for t in range(nt):
                    phase1(t)
                for hargs in hops[:n_hops]:
                    hop(*hargs)
    return _legalize_waits(nc) if legalize else nc


_NC_CACHE = {}


def _get_nc():
    if "nc" not in _NC_CACHE:
        _NC_CACHE["nc"] = build_bass()
    return _NC_CACHE["nc"]


def make_inputs(support0, support1, inputs, state, weight):
    """Host-side layout prep -> per-core in_maps (shared replicated arrays)."""
    xs = np.concatenate(
        [
            np.asarray(inputs, np.float32).reshape(B, N, F // 2),
            np.asarray(state, np.float32).reshape(B, N, F // 2),
        ],
        axis=2,
    )  # [B, N, F]

    w = np.asarray(weight, np.float32).reshape(F, M5, O)
    wv0 = w[:, 0] - w[:, 2] - w[:, 4]
    wcat = np.concatenate(
        [wv0, w[:, 1], 2.0 * w[:, 2], w[:, 3], 2.0 * w[:, 4]], axis=1
    ).astype(NPBF16)  # [128, 320]

    def strip_img(s):
        # [t, p, kt*128+j] = s[t*128+j, kt*128+p]
        r = np.asarray(s, np.float32).astype(NPBF16)
        r = r.reshape(NT, 128, NT, 128).transpose(0, 3, 2, 1)  # [t, p, kt, j]
        return np.ascontiguousarray(r.reshape(NT, 128, N))

    s0i, s1i = strip_img(support0), strip_img(support1)

    in_maps = []
    for c in range(NCORES):
        shard = xs[c * BS:(c + 1) * BS]                # [8b, N, F]
        # t-major SBUF image: x0t[f, t*BS*128 + b*128 + j] = shard[b, t*128+j, f]
        x0t = np.ascontiguousarray(
            shard.reshape(BS, NT, 128, F).transpose(3, 1, 0, 2).reshape(F, BS * N)
        ).astype(NPBF16)
        in_maps.append({"x0t": x0t, "wcat": wcat, "s0t": s0i, "s1t": s1i})
    return in_maps


def postprocess(results, biases):
    full = np.empty((B, N, O), np.float32)
    for c, r in enumerate(results):
        full[c * BS:(c + 1) * BS] = (
            r["out"].reshape(N, BS, O).transpose(1, 0, 2)
        )
    full += np.asarray(biases, np.float32)[None, None, :]
    return full.reshape(B, N * O)


def kernel(support0, support1, inputs, state, weight, biases, output_size=None,
           **run_kwargs):
    nc = _get_nc()
    in_maps = make_inputs(support0, support1, inputs, state, weight)
    res = run_bass_kernel_spmd(nc, in_maps, core_ids=list(range(NCORES)),
                               **run_kwargs)
    out = postprocess(res.results, biases)
    if run_kwargs.get("trace"):
        return out, res
    return out
